# revision 1
# baseline (speedup 1.0000x reference)
# Per-sample 256-bin histogram entropy on trn2 (8 cores, data-parallel over batch).
#
# Algorithm (per core, 8 samples of 786432 f32 each):
#   1. DMA f32 sample into SBUF arena.
#   2. Per-sample min/max: DVE free-dim reduce + GPSIMD partition_all_reduce.
#   3. t = (x + (-min)) * (256/range) in [0, 256] (ACT); hi16 = round(t/16 - 0.5+eps)
#      (= floor(t/16)); v = t - 16*hi16 in [0, 16].
#   4. Step matrices, stored CONTIGUOUSLY per threshold (slab [P, 16, w]):
#      HI[i] = [t >= 16i], LO[j] = [v >= j] as bf16 0/1. Contiguous [P, w] writes
#      keep the DVE in 4x mode (the v1 interleaved-slot layout forced 8-elem
#      strided writes, ~10x slower per op on HW). Thresholds are split across
#      DVE (is_ge), ACT (saturated sigmoid), and GPSIMD (is_ge).
#   5. PE matmuls: for each group of ES=8 elements, operands are strided AP
#      views HI[:, :, g*8:(g+1)*8] -> [P, 128]; accumulate O = HI^T @ LO into
#      PSUM. Diagonal element slots give C[i,j] = #{hi >= i AND lo >= j}.
#   6. Host: 2D difference of C -> 256-bin histogram -> entropy -> mean.
#
# The 2D-cumulative/step trick avoids any floor() on device: [t >= 16i] <=> [floor(t/16) >= i].
import numpy as np

P = 128          # SBUF partitions
NB = 16          # bins per level (16 hi x 16 lo = 256)
ES = 8           # element slots per matmul column block
NCORES = 8
BATCH = 64
SPC = BATCH // NCORES          # samples per core
NPS = 3 * 512 * 512            # elements per sample
FPS = NPS // P                 # free-dim length per sample = 6144


def build_nc(spc=SPC, fps=FPS, w=768, ch=2048, cvt_bias=-0.5 + 2**-16,
             fps_phys=None, reps=1, act_lo=9, act_hi=9,
             xt_bufs=2, slab_bufs=2, tv_bufs=3, dve_t=False, skip_mm=False,
             skip_steps=False, debug_taps=False):
    # Both slabs are element-slot interleaved [P, g, NB*ES] f16 so matmul
    # operands are flat contiguous [P, 128] group slices. f16 (not bf16!)
    # slab dtype keeps step ops in the DVE's packed perf modes: HW-measured
    # per-op cost for [128,768] is_ge f16->f16 is ~0.8us strided vs ~1.4us+
    # for f16->bf16 (mode drop), and ~3.5us strided bf16. ACT sigmoid strided
    # f16 is ~0.67us. GPSIMD tensor_scalar is ~12.4us regardless of layout --
    # never put slab work there.
    # act_*: how many of the 16 lo/hi thresholds (counted from the top) run
    # on ACT (saturated sigmoid); the rest run on DVE (is_ge).
    # cvt_bias: pre-shift before the f32->int16 convert in the floor(t/16)
    # pass. HW converts round-to-nearest -> -0.5+eps gives floor; CoreSim
    # truncates -> pass +eps instead when simulating.
    # fps_phys/reps: benchmarking aids. fps_phys < fps makes the DRAM input
    # physically smaller (chunks read modulo fps_phys; same HBM traffic and
    # compute). reps repeats the whole pipeline to amplify compute vs fixed
    # per-call overhead.
    import concourse.bacc as bacc
    import concourse.mybir as mybir
    import concourse.tile as tile
    from concourse import bass_isa

    if fps_phys is None:
        fps_phys = fps
    assert fps % w == 0 and w % ES == 0 and fps_phys % ch == 0
    g = w // ES                # matmul groups per macro-tile
    nmacro = fps // w
    f32 = mybir.dt.float32
    f16 = mybir.dt.float16
    bf16 = mybir.dt.bfloat16
    i16 = mybir.dt.int16
    Alu = mybir.AluOpType
    Act = mybir.ActivationFunctionType
    X = mybir.AxisListType.X

    nc = bacc.Bacc(None, target_bir_lowering=False, debug=False)
    x_in = nc.declare_dram_parameter("x", [spc, P, fps_phys], f32, isOutput=False)
    c_out = nc.declare_dram_parameter("cmat", [spc, P, P], f32, isOutput=True)
    if debug_taps:
        tt_out = nc.declare_dram_parameter("tt_dbg", [P, w], f16, isOutput=True)
        vv_out = nc.declare_dram_parameter("vv_dbg", [P, w], f16, isOutput=True)
        hi_out = nc.declare_dram_parameter("hi_dbg", [P, NB, w], bf16, isOutput=True)
        lo_out = nc.declare_dram_parameter("lo_dbg", [P, NB, w], bf16, isOutput=True)

    with tile.TileContext(nc) as tc:
        with (
            tc.tile_pool(name="xf", bufs=xt_bufs) as x_pool,
            tc.tile_pool(name="tv", bufs=tv_bufs) as tv_pool,
            tc.tile_pool(name="slab", bufs=slab_bufs) as slab_pool,
            tc.tile_pool(name="small", bufs=2) as small_pool,
            tc.tile_pool(name="co", bufs=2) as co_pool,
            tc.tile_pool(name="const", bufs=1) as const_pool,
            tc.tile_pool(name="psum", bufs=2, space="PSUM") as psum_pool,
        ):
            actb_hi = actb_lo = None
            if act_hi or act_lo:
                actb_hi = [
                    const_pool.tile([P, 1], f32, tag=f"abh{i}", name=f"abh{i}")
                    for i in range(NB)
                ]
                actb_lo = [
                    const_pool.tile([P, 1], f32, tag=f"abl{i}", name=f"abl{i}")
                    for i in range(NB)
                ]
                # thresholds shifted off the fp16 value grids so sigmoid's 0.5
                # at-exact-threshold never fires ([t >= thr] elements all land
                # at sigmoid(+big) = 1.0 exactly)
                for i in range(NB):
                    nc.vector.memset(actb_hi[i][:], -4096.0 * (16.0 * i - 0.06))
                    nc.vector.memset(actb_lo[i][:], -4096.0 * (i - 0.03))
            for rep_s in range(reps * spc):
                s = rep_s % spc
                # ---- phase A: load + min/max ----
                xt = x_pool.tile([P, fps], f32, tag="xt")
                for c in range(0, fps, ch):
                    cp = c % fps_phys
                    nc.sync.dma_start(out=xt[:, c : c + ch], in_=x_in[s, :, cp : cp + ch])
                mx = small_pool.tile([P, 1], f32, tag="mx")
                mn = small_pool.tile([P, 1], f32, tag="mn")
                nc.vector.tensor_reduce(mx[:], xt[:], axis=X, op=Alu.max)
                nc.vector.tensor_reduce(mn[:], xt[:], axis=X, op=Alu.min)
                nmn = small_pool.tile([P, 1], f32, tag="nmn")
                nc.vector.tensor_scalar_mul(nmn[:], mn[:], -1.0)
                # cross-partition: all partitions end up with the global value
                mxr = small_pool.tile([P, 1], f32, tag="mxr")
                nmnr = small_pool.tile([P, 1], f32, tag="nmnr")
                nc.gpsimd.partition_all_reduce(
                    mxr[:], mx[:], channels=P, reduce_op=bass_isa.ReduceOp.max
                )
                nc.gpsimd.partition_all_reduce(
                    nmnr[:], nmn[:], channels=P, reduce_op=bass_isa.ReduceOp.max
                )
                rng = small_pool.tile([P, 1], f32, tag="rng")
                nc.vector.tensor_tensor(rng[:], mxr[:], nmnr[:], op=Alu.add)
                rcp = small_pool.tile([P, 1], f32, tag="rcp")
                nc.vector.reciprocal(rcp[:], rng[:])
                sc = small_pool.tile([P, 1], f32, tag="sc")
                nc.vector.tensor_scalar_mul(sc[:], rcp[:], 256.0)
                nmnsc = small_pool.tile([P, 1], f32, tag="nmnsc")
                nc.vector.tensor_tensor(nmnsc[:], nmnr[:], sc[:], op=Alu.mult)

                # ---- phase B: binning ----
                cm = psum_pool.tile([P, P], f32, tag="cm")
                for m in range(nmacro):
                    xs = xt[:, m * w : (m + 1) * w]
                    tt = tv_pool.tile([P, w], f16, tag="tt")
                    hi16 = tv_pool.tile([P, w], i16, tag="hi16")
                    vv = tv_pool.tile([P, w], f16, tag="vv")
                    # t = (x + nmn) * sc in [0, 256]
                    if dve_t:
                        nc.vector.tensor_scalar(
                            tt[:], xs, nmnr[:], sc[:], op0=Alu.add, op1=Alu.mult
                        )
                    else:
                        # t >= 0 so Abs is identity; Copy rejects AP bias
                        nc.scalar.activation(
                            tt[:], xs, Act.Abs, bias=nmnsc[:], scale=sc[:]
                        )
                    # floor(t/16) via round-nearest int convert. No clamp: only
                    # the x == max element (t = 256) overflows to hi16 = 16,
                    # landing in bin (15,0) instead of (15,15) -- 1 element of
                    # 786432, ~1e-5 relative entropy effect.
                    nc.vector.tensor_scalar(
                        hi16[:], tt[:], 0.0625, cvt_bias, op0=Alu.mult, op1=Alu.add
                    )
                    # v = t - 16*floor(t/16) in [0, 16]
                    nc.vector.scalar_tensor_tensor(
                        out=vv[:], in0=hi16[:], scalar=-16.0, in1=tt[:],
                        op0=Alu.mult, op1=Alu.add,
                    )
                    hi_sl = slab_pool.tile([P, g, NB * ES], f16, tag="hi")
                    lo_sl = slab_pool.tile([P, g, NB * ES], f16, tag="lo")
                    t3 = tt[:].rearrange("p (g e) -> p g e", e=ES)
                    v3 = vv[:].rearrange("p (g e) -> p g e", e=ES)
                    for i in range(NB if not skip_steps else 1):
                        thr_hi = 16.0 * i if i else -1.0
                        thr_lo = float(i) if i else -1.0
                        hi_dst = hi_sl[:, :, ES * i : ES * (i + 1)]
                        lo_dst = lo_sl[:, :, ES * i : ES * (i + 1)]
                        # saturated sigmoid: sigmoid(4096*(t-thr)) is exactly
                        # 0.0/1.0 in f16 outside a ~0.005-wide boundary zone
                        if i >= NB - act_hi:
                            nc.scalar.activation(
                                hi_dst, t3, Act.Sigmoid,
                                bias=actb_hi[i][:], scale=4096.0,
                            )
                        else:
                            nc.vector.tensor_scalar(
                                hi_dst, t3, thr_hi, None, op0=Alu.is_ge
                            )
                        if i >= NB - act_lo:
                            nc.scalar.activation(
                                lo_dst, v3, Act.Sigmoid,
                                bias=actb_lo[i][:], scale=4096.0,
                            )
                        else:
                            nc.vector.tensor_scalar(
                                lo_dst, v3, thr_lo, None, op0=Alu.is_ge
                            )
                    if debug_taps and s == 0 and m == 0:
                        nc.sync.dma_start(out=tt_out[:], in_=tt[:])
                        nc.sync.dma_start(out=vv_out[:], in_=vv[:])
                        nc.sync.dma_start(out=hi_out[:], in_=hi_sl[:])
                        nc.sync.dma_start(out=lo_out[:], in_=lo_sl[:])
                    for gi in range(g if not skip_mm else 1):
                        nc.tensor.matmul(
                            cm[:],
                            hi_sl[:, gi, :],
                            lo_sl[:, gi, :],
                            start=(m == 0 and gi == 0),
                            stop=(m == nmacro - 1 and gi == (g - 1 if not skip_mm else 0)),
                        )
                co = co_pool.tile([P, P], f32, tag="co")
                nc.scalar.activation(co[:], cm[:], Act.Copy)
                nc.sync.dma_start(out=c_out[s], in_=co[:])
    nc.compile()
    return nc


def postprocess(cmats, n_per_sample):
    """cmats: [nsamples, P, P] f32 matmul outputs -> list of entropies (bits)."""
    ents = []
    for O in cmats:
        O4 = O.reshape(NB, ES, NB, ES)
        C2 = np.einsum("iaja->ij", O4)  # sum diagonal element slots
        Cp = np.zeros((NB + 1, NB + 1))
        Cp[:NB, :NB] = C2
        h = Cp[:NB, :NB] - Cp[1:, :NB] - Cp[:NB, 1:] + Cp[1:, 1:]
        hist = h.reshape(NB * NB)
        total = hist.sum()
        p = hist / total
        nz = p > 0
        ents.append(-(p[nz] * np.log2(p[nz])).sum())
    return ents


_NC_CACHE = {}

# Steps split so DVE (~0.8us/op + cvt/stt/reduces) and ACT (~0.67us/op + t)
# finish together; GPSIMD only does the tiny partition_all_reduces.
BEST_CFG = dict(act_lo=9, act_hi=9, w=768)


def kernel(y_pred: np.ndarray) -> np.ndarray:
    from concourse.bass_utils import run_bass_kernel_spmd

    assert y_pred.shape == (BATCH, 3, 512, 512) and y_pred.dtype == np.float32
    x = np.ascontiguousarray(y_pred).reshape(NCORES, SPC, P, FPS)
    in_maps = [{"x": x[c]} for c in range(NCORES)]
    if "nc" not in _NC_CACHE:
        _NC_CACHE["nc"] = build_nc(**BEST_CFG)
    res = run_bass_kernel_spmd(_NC_CACHE["nc"], in_maps, list(range(NCORES))).results
    ents = []
    for c in range(NCORES):
        ents.extend(postprocess(res[c]["cmat"], NPS))
    return np.array(np.mean(ents), dtype=np.float32)


if __name__ == "__main__":
    import reference

    inputs = reference.setup_inputs()
    y = np.asarray(inputs["y_pred"])
    out = kernel(y)
    print("kernel out:", out)

